# revision 9
# baseline (speedup 1.0000x reference)
"""Trainium2 kernel for per-class conditional dense (MoE-style routing).

    out[b] = x[b] @ W[classes[b]] + bias[classes[b]]
    x: [2048, 512] f32, classes: [2048, 1] int, W: [100, 512, 512] f32,
    bias: [100, 512] f32 -> out: [2048, 512] f32

Sharding: expert-parallel across 8 NeuronCores (grouped-GEMM style).
Class c is owned by core c // 13 (13 class slots per core; 8*13 = 104
slots cover the 100 classes, the last 4 slots are dummies). The host
routes each sample to the core owning its class, packing the samples of
each class into a fixed-width (S columns, zero-padded) block of a
transposed activation panel, casts both the panel and the core's weight
slots to bf16, and pre-tiles them into the exact SBUF layout so every
DMA line is one long contiguous descriptor (3-4 KB per partition).

Each core runs a fully static grouped GEMM over its 13 slots: the x
panel is PE-stationary, the bf16 weight rows stream as the moving
operand (one pass per matmul instead of fp32's LOW/HIGH two-pass),
accumulating fp32 in PSUM. Four class slots share one PSUM bank
([4*S=128, U]), so a single ACTIVATE drains 4 classes to a bf16 SBUF
tile which is stored from the Scalar queue. The host scatters the
panel rows back to sample order and adds the bias in fp32.

bf16 halves both the HBM traffic (the binding roofline: ~6.8 MB of
weights per core at ~360 GB/s) and the PE streaming time versus the
fp32 baseline. bf16 rounding of x and W gives ~1e-3 relative error,
well inside the 2e-2 gate.
"""

import sys
import types

import numpy as np

try:
    import concourse.bass as bass
except ImportError:  # pragma: no cover - fallback for bare environments
    for _p in ("/opt/trn_rl_repo", "/root/.axon_site/_ro/trn_rl_repo"):
        if _p not in sys.path:
            sys.path.insert(0, _p)
    import concourse.bass as bass

try:  # pragma: no cover
    import antenv.axon_hooks  # noqa: F401
except ImportError:
    # bass_utils imports this when BASS_TRACE is set; the agent image's
    # antenv lacks it. Register a no-op shim so tracing degrades to a
    # plain (untraced) run instead of crashing.
    _hooks = types.ModuleType("antenv.axon_hooks")
    _hooks.get_axon_ntff_profile_hook = lambda: None
    _hooks.set_axon_ntff_profile_hook = lambda h: None
    sys.modules["antenv.axon_hooks"] = _hooks

import bass_rust
import ml_dtypes
import concourse.tile as tile
from concourse import mybir
from concourse.bass_utils import run_bass_kernel_spmd

B, D, U, C = 2048, 512, 512, 100
NCORES = 8
CPC = 13  # class slots per core (8 * 13 = 104 >= C)
PT = 128  # partition tile
KT = D // PT  # contraction-dim tiles
BF16 = ml_dtypes.bfloat16

_PROG_CACHE = {}
LAST_RESULTS = None  # BassKernelResults of the most recent device run


def _split_multi_waits(nc):
    """Walrus on this image only accepts one sync wait per instruction.

    Tile emits multi-wait instructions (notably the kernel-tail Drain,
    which waits on every live semaphore). Split each extra wait onto a
    same-engine NoOp inserted immediately before the instruction.
    """
    for fn in nc.m.functions:
        for bb in fn.blocks:
            new = []
            changed = False
            for inst in bb.instructions:
                si = inst.sync_info
                waits = list(si.on_wait) if si else []
                if len(waits) > 1:
                    for idx, w in enumerate(waits[:-1]):
                        nop = mybir.InstNoOp(
                            name=f"{inst.name}-waitsplit{idx}", ins=[], outs=[]
                        )
                        nop.engine = inst.engine
                        nop.sync_info = bass_rust.SyncInfo(
                            on_wait=[w], on_update=[]
                        )
                        new.append(nop)
                    inst.sync_info = bass_rust.SyncInfo(
                        on_wait=[waits[-1]], on_update=list(si.on_update)
                    )
                    changed = True
                new.append(inst)
            if changed:
                bb.instructions = new


def _build_program(S):
    """One SPMD program, shared by all 8 cores; per-core data differs.

    Per core: xt [PT, KT*NCOL] bf16 (pre-tiled transposed class-blocked
    activations), wt [CPC, PT, KT*U] bf16 (pre-tiled weight slots)
    -> out [NCOL, U] bf16.

    The x panel is the PE-stationary operand (tiny, so per-matmul
    LDWEIGHTS stays off the critical path) and the bf16 weight rows
    stream through as the moving operand with a 512-wide free dim at
    one pass per matmul. fp32 would stream the same columns twice
    (LOW/HIGH passes) and move twice the HBM bytes.
    """
    f32 = mybir.dt.float32
    bf16 = mybir.dt.bfloat16
    NCOL = CPC * S
    GRP = PT // S  # class slots sharing one PSUM bank / output tile
    OG = -(-CPC // GRP)  # output groups

    nc = bass.Bass()
    xt = nc.dram_tensor("xt", [PT, KT * NCOL], bf16, kind="ExternalInput")
    wt = nc.dram_tensor("wt", [CPC, PT, KT * U], bf16, kind="ExternalInput")
    out = nc.dram_tensor("out", [NCOL, U], bf16, kind="ExternalOutput")

    with tile.TileContext(nc) as tc:
        with (
            tc.tile_pool(name="xp", bufs=1) as xp,
            tc.tile_pool(name="wp", bufs=CPC) as wp,
            tc.tile_pool(name="op", bufs=1) as op,
            tc.tile_pool(name="pp", bufs=3, space="PSUM") as pp,
            tc.tile_pool(name="ap", bufs=1, space="PSUM") as apool,
        ):
            # Per-class weight DMA, also layout-identical: 128 lines of
            # KT*U*2 = 4 KB each, double-buffered CPC deep. The first and
            # last slots are split in halves so the PE can start ~0.7 us
            # earlier and so only half of slot 12 remains to be multiplied
            # after the final HBM byte lands.
            HW = KT * U // 2
            w_ts = []
            for j in range(CPC):
                w_t = wp.tile([PT, KT * U], bf16, tag="w", name=f"w{j}")
                # Each slot in two half-DMAs: 2 KB lines avoid the 4 KB
                # straggler-engine effect, the PE can start on a half, and
                # the i=2 LDWEIGHTS absorbs the second half's wait with no
                # dummy needed.
                nc.sync.dma_start(w_t[:, :HW], wt[j][:, :HW])
                if j == 0:
                    # x panel after w0's first half: the first matmuls
                    # need both, everything later only needs the weights.
                    xt_t = xp.tile([PT, KT * NCOL], bf16, name="x")
                    nc.sync.dma_start(xt_t[:], xt[:, :])
                nc.sync.dma_start(w_t[:, HW:], wt[j][:, HW:])
                w_ts.append(w_t)

            ots = [op.tile([PT, U], bf16, name=f"o{g}") for g in range(OG)]

            # The LDWEIGHTS half of a matmul only supports one sync wait.
            # Per class, a tiny dummy matmul reading only w_t absorbs the
            # weight-DMA wait on the PE engine, so the real matmuls below
            # carry at most the xt-DMA / PSUM-reuse wait. The dummies form
            # one accumulation group into the same scratch element so they
            # do not create PSUM WAW waits among themselves.
            scrps = apool.tile([2, 2], f32, name="abs")

            def dummy(j, col, start=False, stop=False):
                nc.tensor.matmul(
                    scrps[:],
                    w_ts[j][:, col : col + 2],
                    w_ts[j][:, col + 2 : col + 4],
                    start=start,
                    stop=stop,
                    skip_group_check=True,
                )

            stores = []
            for j in range(CPC):
                # A dummy is only needed where the i=0 LDWEIGHTS must carry
                # another wait besides the half-A DMA: class 0 (xt DMA) and
                # the PSUM-bank-reuse classes (ACT completion).
                if j == 0 or j >= GRP * 3:
                    dummy(j, 0, start=(j == 0), stop=(j == CPC - 1))
                g, r = divmod(j, GRP)
                if r == 0:
                    ps = pp.tile([PT, U], f32, tag="ps", name=f"ps{g}")
                for i in range(KT):
                    nc.tensor.matmul(
                        ps[S * r : S * r + S, :],
                        xt_t[:, i * NCOL + S * j : i * NCOL + S * (j + 1)],
                        w_ts[j][:, i * U : (i + 1) * U],
                        start=(i == 0),
                        stop=(i == KT - 1),
                        # PE-array column offset = PSUM partition offset;
                        # auto-infer rejects 96, so pass it explicitly.
                        tile_position=(0, S * r),
                    )
                # Drain the PSUM bank once its last class finishes: one
                # ACTIVATE casts 4 classes fp32->bf16 into the output tile.
                if r == GRP - 1 or j == CPC - 1:
                    rows = min(PT, NCOL - PT * g)
                    nc.scalar.copy(ots[g][:rows, :], ps[:rows, :])
                    stores.append((g, rows))
            # All stores at the end: HBM stays read-only while the weight
            # stream is in flight (read/write turnaround stalls the DMA
            # engines mid-stream), and the Scalar queue issues them
            # back-to-back once the last PSUM bank drains.
            for g, rows in stores:
                nc.scalar.dma_start(
                    out[PT * g : PT * g + rows, :], ots[g][:rows, :]
                )
    _split_multi_waits(nc)
    return nc


def kernel(x, classes, kernel, bias):
    global LAST_RESULTS
    x = np.asarray(x, dtype=np.float32)
    W = np.asarray(kernel, dtype=np.float32)
    bias_np = np.asarray(bias, dtype=np.float32)
    cls = np.asarray(classes).reshape(-1).astype(np.int64)

    counts = np.bincount(cls, minlength=C)
    # Fixed column width per class slot; multiple of 8 for DMA alignment.
    S = int(max(32, -(-counts.max() // 8) * 8))
    if S not in _PROG_CACHE:
        _PROG_CACHE[S] = _build_program(S)
    nc = _PROG_CACHE[S]
    NCOL = CPC * S

    order = np.argsort(cls, kind="stable")
    starts = np.zeros(C + 1, np.int64)
    np.cumsum(counts[:C], out=starts[1:])
    rows_by_class = [order[starts[c] : starts[c + 1]] for c in range(C)]

    # Weight slots, pre-tiled to the SBUF layout: [slot, p, i*U+u] holds
    # W[c, i*128+p, u] so each DMA line is 4 KB contiguous.
    W_t = np.ascontiguousarray(
        W.reshape(C, KT, PT, U).transpose(0, 2, 1, 3).reshape(C, PT, KT * U)
    ).astype(BF16)

    in_maps = []
    for m in range(NCORES):
        xt_m = np.zeros((D, NCOL), np.float32)
        for j in range(CPC):
            c = m * CPC + j
            if c >= C:
                continue
            r = rows_by_class[c]
            if r.size:
                xt_m[:, S * j : S * j + r.size] = x[r].T
        # Pre-tile x panel: [p, i*NCOL + c] = xt[i*128+p, c].
        xt_dev = np.ascontiguousarray(
            xt_m.reshape(KT, PT, NCOL).transpose(1, 0, 2).reshape(PT, KT * NCOL)
        ).astype(BF16)
        if (m + 1) * CPC <= C:
            wt_m = W_t[m * CPC : (m + 1) * CPC]
        else:
            n_real = C - m * CPC
            wt_m = np.concatenate([W_t[m * CPC : C], W_t[: CPC - n_real]], axis=0)
        in_maps.append({"xt": xt_dev, "wt": np.ascontiguousarray(wt_m)})

    res = run_bass_kernel_spmd(nc, in_maps, list(range(NCORES)))
    LAST_RESULTS = res

    out = np.empty((B, U), np.float32)
    for m in range(NCORES):
        panel = np.asarray(res.results[m]["out"]).astype(np.float32)
        for j in range(CPC):
            c = m * CPC + j
            if c >= C:
                continue
            r = rows_by_class[c]
            if r.size:
                out[r] = panel[S * j : S * j + r.size] + bias_np[c]
    return out


# revision 11
# speedup vs baseline: 1.1091x; 1.1091x over previous
"""Trainium2 kernel for per-class conditional dense (MoE-style routing).

    out[b] = x[b] @ W[classes[b]] + bias[classes[b]]
    x: [2048, 512] f32, classes: [2048, 1] int, W: [100, 512, 512] f32,
    bias: [100, 512] f32 -> out: [2048, 512] f32

Sharding: expert-parallel across 8 NeuronCores (grouped-GEMM style).
Class c is owned by core c // 13 (13 class slots per core; 8*13 = 104
slots cover the 100 classes, the last 4 slots are dummies). The host
routes each sample to the core owning its class, packing the samples of
each class into a fixed-width (S columns, zero-padded) block of a
transposed activation panel, casts both the panel and the core's weight
slots to bf16, and pre-tiles them into the exact SBUF layout so every
DMA line is one long contiguous descriptor (3-4 KB per partition).

Each core runs a fully static grouped GEMM over its 13 slots: the x
panel is PE-stationary, the bf16 weight rows stream as the moving
operand (one pass per matmul instead of fp32's LOW/HIGH two-pass),
accumulating fp32 in PSUM. Four class slots share one PSUM bank
([4*S=128, U]), so a single ACTIVATE drains 4 classes to a bf16 SBUF
tile which is stored from the Scalar queue. The host scatters the
panel rows back to sample order and adds the bias in fp32.

bf16 halves both the HBM traffic (the binding roofline: ~6.8 MB of
weights per core at ~360 GB/s) and the PE streaming time versus the
fp32 baseline. bf16 rounding of x and W gives ~1e-3 relative error,
well inside the 2e-2 gate.
"""

import sys
import types

import numpy as np

try:
    import concourse.bass as bass
except ImportError:  # pragma: no cover - fallback for bare environments
    for _p in ("/opt/trn_rl_repo", "/root/.axon_site/_ro/trn_rl_repo"):
        if _p not in sys.path:
            sys.path.insert(0, _p)
    import concourse.bass as bass

try:  # pragma: no cover
    import antenv.axon_hooks  # noqa: F401
except ImportError:
    # bass_utils imports this when BASS_TRACE is set; the agent image's
    # antenv lacks it. Register a no-op shim so tracing degrades to a
    # plain (untraced) run instead of crashing.
    _hooks = types.ModuleType("antenv.axon_hooks")
    _hooks.get_axon_ntff_profile_hook = lambda: None
    _hooks.set_axon_ntff_profile_hook = lambda h: None
    sys.modules["antenv.axon_hooks"] = _hooks

import bass_rust
import ml_dtypes
import concourse.tile as tile
from concourse import mybir
from concourse.bass_utils import run_bass_kernel_spmd

B, D, U, C = 2048, 512, 512, 100
NCORES = 8
CPC = 13  # class slots per core (8 * 13 = 104 >= C)
PT = 128  # partition tile
KT = D // PT  # contraction-dim tiles
BF16 = ml_dtypes.bfloat16

_PROG_CACHE = {}
LAST_RESULTS = None  # BassKernelResults of the most recent device run


def _split_multi_waits(nc):
    """Walrus on this image only accepts one sync wait per instruction.

    Tile emits multi-wait instructions (notably the kernel-tail Drain,
    which waits on every live semaphore). Split each extra wait onto a
    same-engine NoOp inserted immediately before the instruction.
    """
    for fn in nc.m.functions:
        for bb in fn.blocks:
            new = []
            changed = False
            for inst in bb.instructions:
                si = inst.sync_info
                waits = list(si.on_wait) if si else []
                if len(waits) > 1:
                    for idx, w in enumerate(waits[:-1]):
                        nop = mybir.InstNoOp(
                            name=f"{inst.name}-waitsplit{idx}", ins=[], outs=[]
                        )
                        nop.engine = inst.engine
                        nop.sync_info = bass_rust.SyncInfo(
                            on_wait=[w], on_update=[]
                        )
                        new.append(nop)
                    inst.sync_info = bass_rust.SyncInfo(
                        on_wait=[waits[-1]], on_update=list(si.on_update)
                    )
                    changed = True
                new.append(inst)
            if changed:
                bb.instructions = new


def _build_program(S):
    """One SPMD program, shared by all 8 cores; per-core data differs.

    Per core: xt [PT, KT*NCOL] bf16 (pre-tiled transposed class-blocked
    activations), wt [CPC, PT, KT*U] bf16 (pre-tiled weight slots)
    -> out [NCOL, U] bf16.

    The x panel is the PE-stationary operand (tiny, so per-matmul
    LDWEIGHTS stays off the critical path) and the bf16 weight rows
    stream through as the moving operand with a 512-wide free dim at
    one pass per matmul. fp32 would stream the same columns twice
    (LOW/HIGH passes) and move twice the HBM bytes.
    """
    f32 = mybir.dt.float32
    bf16 = mybir.dt.bfloat16
    NCOL = CPC * S
    GRP = PT // S  # class slots sharing one PSUM bank / output tile
    OG = -(-CPC // GRP)  # output groups

    nc = bass.Bass()
    xt = nc.dram_tensor("xt", [PT, KT * NCOL], bf16, kind="ExternalInput")
    wt = nc.dram_tensor("wt", [CPC, PT, KT * U], bf16, kind="ExternalInput")
    out = nc.dram_tensor("out", [NCOL, U], bf16, kind="ExternalOutput")

    with tile.TileContext(nc) as tc:
        with (
            tc.tile_pool(name="xp", bufs=1) as xp,
            tc.tile_pool(name="wp", bufs=CPC) as wp,
            tc.tile_pool(name="op", bufs=1) as op,
            tc.tile_pool(name="pp", bufs=3, space="PSUM") as pp,
            tc.tile_pool(name="ap", bufs=1, space="PSUM") as apool,
        ):
            # Per-class weight DMA, also layout-identical: 128 lines of
            # KT*U*2 = 4 KB each, double-buffered CPC deep. The first and
            # last slots are split in halves so the PE can start ~0.7 us
            # earlier and so only half of slot 12 remains to be multiplied
            # after the final HBM byte lands.
            HW = KT * U // 2
            split = {0, CPC - 1}
            w_ts = []
            for j in range(CPC):
                w_t = wp.tile([PT, KT * U], bf16, tag="w", name=f"w{j}")
                # Whole slots move as 128 x 4 KB lines (fastest per-engine
                # rate). The first and last slots go in halves so the PE
                # can start ~0.7 us earlier and only half of the final
                # slot's matmuls trail the last HBM byte.
                if j in split:
                    nc.sync.dma_start(w_t[:, :HW], wt[j][:, :HW])
                else:
                    nc.sync.dma_start(w_t[:], wt[j])
                if j == 0:
                    # x panel after w0's first half: the first matmuls
                    # need both, everything later only needs the weights.
                    xt_t = xp.tile([PT, KT * NCOL], bf16, name="x")
                    nc.sync.dma_start(xt_t[:], xt[:, :])
                if j in split:
                    nc.sync.dma_start(w_t[:, HW:], wt[j][:, HW:])
                w_ts.append(w_t)

            ots = [op.tile([PT, U], bf16, name=f"o{g}") for g in range(OG)]

            # The LDWEIGHTS half of a matmul only supports one sync wait.
            # Per class, a tiny dummy matmul reading only w_t absorbs the
            # weight-DMA wait on the PE engine, so the real matmuls below
            # carry at most the xt-DMA / PSUM-reuse wait. The dummies form
            # one accumulation group into the same scratch element so they
            # do not create PSUM WAW waits among themselves.
            scrps = apool.tile([2, 2], f32, name="abs")

            def dummy(j, col, start=False, stop=False):
                nc.tensor.matmul(
                    scrps[:],
                    w_ts[j][:, col : col + 2],
                    w_ts[j][:, col + 2 : col + 4],
                    start=start,
                    stop=stop,
                    skip_group_check=True,
                )

            stores = []
            for j in range(CPC):
                # A dummy is only needed where the i=0 LDWEIGHTS must carry
                # another wait besides the half-A DMA: class 0 (xt DMA) and
                # the PSUM-bank-reuse classes (ACT completion).
                if j == 0 or j >= GRP * 3:
                    dummy(j, 0, start=(j == 0), stop=(j == CPC - 1))
                g, r = divmod(j, GRP)
                if r == 0:
                    ps = pp.tile([PT, U], f32, tag="ps", name=f"ps{g}")
                for i in range(KT):
                    nc.tensor.matmul(
                        ps[S * r : S * r + S, :],
                        xt_t[:, i * NCOL + S * j : i * NCOL + S * (j + 1)],
                        w_ts[j][:, i * U : (i + 1) * U],
                        start=(i == 0),
                        stop=(i == KT - 1),
                        # PE-array column offset = PSUM partition offset;
                        # auto-infer rejects 96, so pass it explicitly.
                        tile_position=(0, S * r),
                    )
                # Drain the PSUM bank once its last class finishes: one
                # ACTIVATE casts 4 classes fp32->bf16 into the output tile.
                if r == GRP - 1 or j == CPC - 1:
                    rows = min(PT, NCOL - PT * g)
                    if j == CPC - 1:
                        # Final group is tail-critical: split the drain
                        # between Scalar and the otherwise-idle Vector
                        # engine so the two halves convert in parallel.
                        U2 = U // 2
                        nc.scalar.copy(ots[g][:rows, :U2], ps[:rows, :U2])
                        nc.vector.tensor_scalar_add(
                            ots[g][:rows, U2:], ps[:rows, U2:], 0.0
                        )
                    else:
                        nc.scalar.copy(ots[g][:rows, :], ps[:rows, :])
                    stores.append((g, rows))
            # Early groups store from the Scalar queue mid-stream; the
            # final group stores from the Sync queue, which is empty by
            # then and moves the data immediately (the Scalar queue
            # trickles at ~35 GB/s when cold).
            for g, rows in stores[:-1]:
                nc.scalar.dma_start(
                    out[PT * g : PT * g + rows, :], ots[g][:rows, :]
                )
            g, rows = stores[-1]
            U2 = U // 2
            nc.sync.dma_start(
                out[PT * g : PT * g + rows, :U2], ots[g][:rows, :U2]
            )
            nc.sync.dma_start(
                out[PT * g : PT * g + rows, U2:], ots[g][:rows, U2:]
            )
    _split_multi_waits(nc)
    return nc


def kernel(x, classes, kernel, bias):
    global LAST_RESULTS
    x = np.asarray(x, dtype=np.float32)
    W = np.asarray(kernel, dtype=np.float32)
    bias_np = np.asarray(bias, dtype=np.float32)
    cls = np.asarray(classes).reshape(-1).astype(np.int64)

    counts = np.bincount(cls, minlength=C)
    # Fixed column width per class slot; multiple of 8 for DMA alignment.
    S = int(max(32, -(-counts.max() // 8) * 8))
    if S not in _PROG_CACHE:
        _PROG_CACHE[S] = _build_program(S)
    nc = _PROG_CACHE[S]
    NCOL = CPC * S

    order = np.argsort(cls, kind="stable")
    starts = np.zeros(C + 1, np.int64)
    np.cumsum(counts[:C], out=starts[1:])
    rows_by_class = [order[starts[c] : starts[c + 1]] for c in range(C)]

    # Weight slots, pre-tiled to the SBUF layout: [slot, p, i*U+u] holds
    # W[c, i*128+p, u] so each DMA line is 4 KB contiguous.
    W_t = np.ascontiguousarray(
        W.reshape(C, KT, PT, U).transpose(0, 2, 1, 3).reshape(C, PT, KT * U)
    ).astype(BF16)

    in_maps = []
    for m in range(NCORES):
        xt_m = np.zeros((D, NCOL), np.float32)
        for j in range(CPC):
            c = m * CPC + j
            if c >= C:
                continue
            r = rows_by_class[c]
            if r.size:
                xt_m[:, S * j : S * j + r.size] = x[r].T
        # Pre-tile x panel: [p, i*NCOL + c] = xt[i*128+p, c].
        xt_dev = np.ascontiguousarray(
            xt_m.reshape(KT, PT, NCOL).transpose(1, 0, 2).reshape(PT, KT * NCOL)
        ).astype(BF16)
        if (m + 1) * CPC <= C:
            wt_m = W_t[m * CPC : (m + 1) * CPC]
        else:
            n_real = C - m * CPC
            wt_m = np.concatenate([W_t[m * CPC : C], W_t[: CPC - n_real]], axis=0)
        in_maps.append({"xt": xt_dev, "wt": np.ascontiguousarray(wt_m)})

    res = run_bass_kernel_spmd(nc, in_maps, list(range(NCORES)))
    LAST_RESULTS = res

    out = np.empty((B, U), np.float32)
    for m in range(NCORES):
        panel = np.asarray(res.results[m]["out"]).astype(np.float32)
        for j in range(CPC):
            c = m * CPC + j
            if c >= C:
                continue
            r = rows_by_class[c]
            if r.size:
                out[r] = panel[S * j : S * j + r.size] + bias_np[c]
    return out


# revision 16
# speedup vs baseline: 1.1385x; 1.0265x over previous
"""Trainium2 kernel for per-class conditional dense (MoE-style routing).

    out[b] = x[b] @ W[classes[b]] + bias[classes[b]]
    x: [2048, 512] f32, classes: [2048, 1] int, W: [100, 512, 512] f32,
    bias: [100, 512] f32 -> out: [2048, 512] f32

Sharding: expert-parallel across 8 NeuronCores (grouped-GEMM style).
Class c is owned by core c // 13 (13 class slots per core; 8*13 = 104
slots cover the 100 classes, the last 4 slots are dummies). The host
routes each sample to the core owning its class, packing the samples of
each class into a fixed-width (S columns, zero-padded) block of a
transposed activation panel, casts both the panel and the core's weight
slots to bf16, and pre-tiles them into the exact SBUF layout so every
DMA line is one long contiguous descriptor (3-4 KB per partition).

Each core runs a fully static grouped GEMM over its 13 slots: the x
panel is PE-stationary, the bf16 weight rows stream as the moving
operand (one pass per matmul instead of fp32's LOW/HIGH two-pass),
accumulating fp32 in PSUM. Four class slots share one PSUM bank
([4*S=128, U]), so a single ACTIVATE drains 4 classes to a bf16 SBUF
tile which is stored from the Scalar queue. The host scatters the
panel rows back to sample order and adds the bias in fp32.

bf16 halves both the HBM traffic (the binding roofline: ~6.8 MB of
weights per core at ~360 GB/s) and the PE streaming time versus the
fp32 baseline. bf16 rounding of x and W gives ~1e-3 relative error,
well inside the 2e-2 gate.
"""

import sys
import types

import numpy as np

try:
    import concourse.bass as bass
except ImportError:  # pragma: no cover - fallback for bare environments
    for _p in ("/opt/trn_rl_repo", "/root/.axon_site/_ro/trn_rl_repo"):
        if _p not in sys.path:
            sys.path.insert(0, _p)
    import concourse.bass as bass

try:  # pragma: no cover
    import antenv.axon_hooks  # noqa: F401
except ImportError:
    # bass_utils imports this when BASS_TRACE is set; the agent image's
    # antenv lacks it. Register a no-op shim so tracing degrades to a
    # plain (untraced) run instead of crashing.
    _hooks = types.ModuleType("antenv.axon_hooks")
    _hooks.get_axon_ntff_profile_hook = lambda: None
    _hooks.set_axon_ntff_profile_hook = lambda h: None
    sys.modules["antenv.axon_hooks"] = _hooks

import bass_rust
import ml_dtypes
import concourse.tile as tile
from concourse import mybir
from concourse.bass_utils import run_bass_kernel_spmd

B, D, U, C = 2048, 512, 512, 100
NCORES = 8
CPC = 13  # class slots per core (8 * 13 = 104 >= C)
PT = 128  # partition tile
KT = D // PT  # contraction-dim tiles
BF16 = ml_dtypes.bfloat16

_PROG_CACHE = {}
LAST_RESULTS = None  # BassKernelResults of the most recent device run


def _split_multi_waits(nc):
    """Walrus on this image only accepts one sync wait per instruction.

    Tile emits multi-wait instructions (notably the kernel-tail Drain,
    which waits on every live semaphore). Split each extra wait onto a
    same-engine NoOp inserted immediately before the instruction.
    """
    for fn in nc.m.functions:
        for bb in fn.blocks:
            new = []
            changed = False
            for inst in bb.instructions:
                si = inst.sync_info
                waits = list(si.on_wait) if si else []
                if len(waits) > 1:
                    for idx, w in enumerate(waits[:-1]):
                        nop = mybir.InstNoOp(
                            name=f"{inst.name}-waitsplit{idx}", ins=[], outs=[]
                        )
                        nop.engine = inst.engine
                        nop.sync_info = bass_rust.SyncInfo(
                            on_wait=[w], on_update=[]
                        )
                        new.append(nop)
                    inst.sync_info = bass_rust.SyncInfo(
                        on_wait=[waits[-1]], on_update=list(si.on_update)
                    )
                    changed = True
                new.append(inst)
            if changed:
                bb.instructions = new


def _build_program(S):
    """One SPMD program, shared by all 8 cores; per-core data differs.

    Per core: xt [PT, KT*NCOL] bf16 (pre-tiled transposed class-blocked
    activations), wt [CPC, PT, KT*U] bf16 (pre-tiled weight slots)
    -> out [NCOL, U] bf16.

    The x panel is the PE-stationary operand (tiny, so per-matmul
    LDWEIGHTS stays off the critical path) and the bf16 weight rows
    stream through as the moving operand with a 512-wide free dim at
    one pass per matmul. fp32 would stream the same columns twice
    (LOW/HIGH passes) and move twice the HBM bytes.
    """
    f32 = mybir.dt.float32
    bf16 = mybir.dt.bfloat16
    NCOL = CPC * S
    GRP = PT // S  # class slots sharing one PSUM bank / output tile
    OG = -(-CPC // GRP)  # output groups

    nc = bass.Bass()
    xt = nc.dram_tensor("xt", [PT, KT * NCOL], bf16, kind="ExternalInput")
    wt = nc.dram_tensor("wt", [CPC, PT, KT * U], bf16, kind="ExternalInput")
    out = nc.dram_tensor("out", [NCOL, U], bf16, kind="ExternalOutput")

    with tile.TileContext(nc) as tc:
        with (
            tc.tile_pool(name="xp", bufs=1) as xp,
            tc.tile_pool(name="wp", bufs=CPC) as wp,
            tc.tile_pool(name="op", bufs=1) as op,
            tc.tile_pool(name="pp", bufs=3, space="PSUM") as pp,
            tc.tile_pool(name="ap", bufs=1, space="PSUM") as apool,
        ):
            # Per-class weight DMA, also layout-identical: 128 lines of
            # KT*U*2 = 4 KB each, double-buffered CPC deep. The first and
            # last slots are split in halves so the PE can start ~0.7 us
            # earlier and so only half of slot 12 remains to be multiplied
            # after the final HBM byte lands.
            HW = KT * U // 2
            split = {0, CPC - 1}
            w_ts = []
            for j in range(CPC):
                w_t = wp.tile([PT, KT * U], bf16, tag="w", name=f"w{j}")
                # Whole slots move as 128 x 4 KB lines (fastest per-engine
                # rate). The first and last slots go in halves so the PE
                # can start ~0.7 us earlier and only half of the final
                # slot's matmuls trail the last HBM byte.
                if j in split:
                    nc.sync.dma_start(w_t[:, :HW], wt[j][:, :HW])
                else:
                    nc.sync.dma_start(w_t[:], wt[j])
                if j == 0:
                    # x panel after w0's first half: the first matmuls
                    # need both, everything later only needs the weights.
                    xt_t = xp.tile([PT, KT * NCOL], bf16, name="x")
                    nc.sync.dma_start(xt_t[:], xt[:, :])
                if j in split:
                    nc.sync.dma_start(w_t[:, HW:], wt[j][:, HW:])
                w_ts.append(w_t)

            ots = [op.tile([PT, U], bf16, name=f"o{g}") for g in range(OG)]

            # The LDWEIGHTS half of a matmul only supports one sync wait.
            # Per class, a tiny dummy matmul reading only w_t absorbs the
            # weight-DMA wait on the PE engine, so the real matmuls below
            # carry at most the xt-DMA / PSUM-reuse wait. The dummies form
            # one accumulation group into the same scratch element so they
            # do not create PSUM WAW waits among themselves.
            scrps = apool.tile([2, 2], f32, name="abs")

            def dummy(j, col, start=False, stop=False):
                nc.tensor.matmul(
                    scrps[:],
                    w_ts[j][:, col : col + 2],
                    w_ts[j][:, col + 2 : col + 4],
                    start=start,
                    stop=stop,
                    skip_group_check=True,
                )

            stores = []
            for j in range(CPC):
                # A dummy is only needed where the i=0 LDWEIGHTS must carry
                # another wait besides the half-A DMA: class 0 (xt DMA) and
                # the PSUM-bank-reuse classes (ACT completion).
                if j == 0 or j >= GRP * 3:
                    dummy(j, 0, start=(j == 0), stop=(j == CPC - 1))
                g, r = divmod(j, GRP)
                if r == 0:
                    ps = pp.tile([PT, U], f32, tag="ps", name=f"ps{g}")
                for i in range(KT):
                    nc.tensor.matmul(
                        ps[S * r : S * r + S, :],
                        xt_t[:, i * NCOL + S * j : i * NCOL + S * (j + 1)],
                        w_ts[j][:, i * U : (i + 1) * U],
                        start=(i == 0),
                        stop=(i == KT - 1),
                        # PE-array column offset = PSUM partition offset;
                        # auto-infer rejects 96, so pass it explicitly.
                        tile_position=(0, S * r),
                    )
                # Drain the PSUM bank once its last class finishes: one
                # ACTIVATE casts 4 classes fp32->bf16 into the output tile.
                if r == GRP - 1 or j == CPC - 1:
                    rows = min(PT, NCOL - PT * g)
                    nc.scalar.copy(ots[g][:rows, :], ps[:rows, :])
                    stores.append((g, rows))
            # Early groups store from the Scalar queue mid-stream; the
            # final group stores from the Sync queue, which is empty by
            # then and moves the data immediately (the Scalar queue
            # trickles at ~35 GB/s when cold). Re-storing groups 0-1 from
            # the Sync queue first is an idempotent warm-up: their packets
            # sit in the queue behind the weight stream and execute right
            # as it drains, keeping the DMA engines awake until the final
            # store's descriptors arrive.
            for g, rows in stores[:-1]:
                nc.scalar.dma_start(
                    out[PT * g : PT * g + rows, :], ots[g][:rows, :]
                )
            for g, rows in stores[:2]:
                nc.sync.dma_start(
                    out[PT * g : PT * g + rows, :], ots[g][:rows, :]
                )
            gl, rows = stores[-1]
            nc.sync.dma_start(
                out[PT * gl : PT * gl + rows, :], ots[gl][:rows, :]
            )
    _split_multi_waits(nc)
    return nc


def kernel(x, classes, kernel, bias):
    global LAST_RESULTS
    x = np.asarray(x, dtype=np.float32)
    W = np.asarray(kernel, dtype=np.float32)
    bias_np = np.asarray(bias, dtype=np.float32)
    cls = np.asarray(classes).reshape(-1).astype(np.int64)

    counts = np.bincount(cls, minlength=C)
    # Fixed column width per class slot; multiple of 8 for DMA alignment.
    S = int(max(32, -(-counts.max() // 8) * 8))
    if S not in _PROG_CACHE:
        _PROG_CACHE[S] = _build_program(S)
    nc = _PROG_CACHE[S]
    NCOL = CPC * S

    order = np.argsort(cls, kind="stable")
    starts = np.zeros(C + 1, np.int64)
    np.cumsum(counts[:C], out=starts[1:])
    rows_by_class = [order[starts[c] : starts[c + 1]] for c in range(C)]

    # Weight slots, pre-tiled to the SBUF layout: [slot, p, i*U+u] holds
    # W[c, i*128+p, u] so each DMA line is 4 KB contiguous.
    W_t = np.ascontiguousarray(
        W.reshape(C, KT, PT, U).transpose(0, 2, 1, 3).reshape(C, PT, KT * U)
    ).astype(BF16)

    in_maps = []
    for m in range(NCORES):
        xt_m = np.zeros((D, NCOL), np.float32)
        for j in range(CPC):
            c = m * CPC + j
            if c >= C:
                continue
            r = rows_by_class[c]
            if r.size:
                xt_m[:, S * j : S * j + r.size] = x[r].T
        # Pre-tile x panel: [p, i*NCOL + c] = xt[i*128+p, c].
        xt_dev = np.ascontiguousarray(
            xt_m.reshape(KT, PT, NCOL).transpose(1, 0, 2).reshape(PT, KT * NCOL)
        ).astype(BF16)
        if (m + 1) * CPC <= C:
            wt_m = W_t[m * CPC : (m + 1) * CPC]
        else:
            n_real = C - m * CPC
            wt_m = np.concatenate([W_t[m * CPC : C], W_t[: CPC - n_real]], axis=0)
        in_maps.append({"xt": xt_dev, "wt": np.ascontiguousarray(wt_m)})

    res = run_bass_kernel_spmd(nc, in_maps, list(range(NCORES)))
    LAST_RESULTS = res

    out = np.empty((B, U), np.float32)
    for m in range(NCORES):
        panel = np.asarray(res.results[m]["out"]).astype(np.float32)
        for j in range(CPC):
            c = m * CPC + j
            if c >= C:
                continue
            r = rows_by_class[c]
            if r.size:
                out[r] = panel[S * j : S * j + r.size] + bias_np[c]
    return out


# revision 19
# speedup vs baseline: 1.1661x; 1.0242x over previous
"""Trainium2 kernel for per-class conditional dense (MoE-style routing).

    out[b] = x[b] @ W[classes[b]] + bias[classes[b]]
    x: [2048, 512] f32, classes: [2048, 1] int, W: [100, 512, 512] f32,
    bias: [100, 512] f32 -> out: [2048, 512] f32

Sharding: expert-parallel across 8 NeuronCores (grouped-GEMM style).
Class c is owned by core c // 13 (13 class slots per core; 8*13 = 104
slots cover the 100 classes, the last 4 slots are dummies). The host
routes each sample to the core owning its class, packing the samples of
each class into a fixed-width (S columns, zero-padded) block of a
transposed activation panel, casts both the panel and the core's weight
slots to bf16, and pre-tiles them into the exact SBUF layout so every
DMA line is one long contiguous descriptor (3-4 KB per partition).

Each core runs a fully static grouped GEMM over its 13 slots: the x
panel is PE-stationary, the bf16 weight rows stream as the moving
operand (one pass per matmul instead of fp32's LOW/HIGH two-pass),
accumulating fp32 in PSUM. Four class slots share one PSUM bank
([4*S=128, U]), so a single ACTIVATE drains 4 classes to a bf16 SBUF
tile which is stored from the Scalar queue. The host scatters the
panel rows back to sample order and adds the bias in fp32.

bf16 halves both the HBM traffic (the binding roofline: ~6.8 MB of
weights per core at ~360 GB/s) and the PE streaming time versus the
fp32 baseline. bf16 rounding of x and W gives ~1e-3 relative error,
well inside the 2e-2 gate.
"""

import sys
import types

import numpy as np

try:
    import concourse.bass as bass
except ImportError:  # pragma: no cover - fallback for bare environments
    for _p in ("/opt/trn_rl_repo", "/root/.axon_site/_ro/trn_rl_repo"):
        if _p not in sys.path:
            sys.path.insert(0, _p)
    import concourse.bass as bass

try:  # pragma: no cover
    import antenv.axon_hooks  # noqa: F401
except ImportError:
    # bass_utils imports this when BASS_TRACE is set; the agent image's
    # antenv lacks it. Register a no-op shim so tracing degrades to a
    # plain (untraced) run instead of crashing.
    _hooks = types.ModuleType("antenv.axon_hooks")
    _hooks.get_axon_ntff_profile_hook = lambda: None
    _hooks.set_axon_ntff_profile_hook = lambda h: None
    sys.modules["antenv.axon_hooks"] = _hooks

import bass_rust
import ml_dtypes
import concourse.tile as tile
from concourse import mybir
from concourse.bass_utils import run_bass_kernel_spmd

B, D, U, C = 2048, 512, 512, 100
NCORES = 8
CPC = 13  # class slots per core (8 * 13 = 104 >= C)
PT = 128  # partition tile
KT = D // PT  # contraction-dim tiles
BF16 = ml_dtypes.bfloat16

_PROG_CACHE = {}
LAST_RESULTS = None  # BassKernelResults of the most recent device run


def _split_multi_waits(nc):
    """Walrus on this image only accepts one sync wait per instruction.

    Tile emits multi-wait instructions (notably the kernel-tail Drain,
    which waits on every live semaphore). Split each extra wait onto a
    same-engine NoOp inserted immediately before the instruction.
    """
    for fn in nc.m.functions:
        for bb in fn.blocks:
            new = []
            changed = False
            for inst in bb.instructions:
                si = inst.sync_info
                waits = list(si.on_wait) if si else []
                if len(waits) > 1:
                    for idx, w in enumerate(waits[:-1]):
                        nop = mybir.InstNoOp(
                            name=f"{inst.name}-waitsplit{idx}", ins=[], outs=[]
                        )
                        nop.engine = inst.engine
                        nop.sync_info = bass_rust.SyncInfo(
                            on_wait=[w], on_update=[]
                        )
                        new.append(nop)
                    inst.sync_info = bass_rust.SyncInfo(
                        on_wait=[waits[-1]], on_update=list(si.on_update)
                    )
                    changed = True
                new.append(inst)
            if changed:
                bb.instructions = new


def _build_program(S):
    """One SPMD program, shared by all 8 cores; per-core data differs.

    Per core: xt [PT, KT*NCOL] bf16 (pre-tiled transposed class-blocked
    activations), wt [CPC, PT, KT*U] bf16 (pre-tiled weight slots)
    -> out [NCOL, U] bf16.

    The x panel is the PE-stationary operand (tiny, so per-matmul
    LDWEIGHTS stays off the critical path) and the bf16 weight rows
    stream through as the moving operand with a 512-wide free dim at
    one pass per matmul. fp32 would stream the same columns twice
    (LOW/HIGH passes) and move twice the HBM bytes.
    """
    f32 = mybir.dt.float32
    bf16 = mybir.dt.bfloat16
    NCOL = CPC * S
    GRP = PT // S  # class slots sharing one PSUM bank / output tile
    OG = -(-CPC // GRP)  # output groups

    nc = bass.Bass()
    xt = nc.dram_tensor("xt", [PT, KT * NCOL], bf16, kind="ExternalInput")
    wt = nc.dram_tensor("wt", [CPC, PT, KT * U], bf16, kind="ExternalInput")
    out = nc.dram_tensor("out", [NCOL, U], bf16, kind="ExternalOutput")

    with tile.TileContext(nc) as tc:
        with (
            tc.tile_pool(name="xp", bufs=1) as xp,
            tc.tile_pool(name="wp", bufs=CPC) as wp,
            tc.tile_pool(name="op", bufs=1) as op,
            tc.tile_pool(name="pp", bufs=3, space="PSUM") as pp,
            tc.tile_pool(name="ap", bufs=1, space="PSUM") as apool,
        ):
            # Per-class weight DMA, also layout-identical: 128 lines of
            # KT*U*2 = 4 KB each, double-buffered CPC deep. The first and
            # last slots are split in halves so the PE can start ~0.7 us
            # earlier and so only half of slot 12 remains to be multiplied
            # after the final HBM byte lands.
            HW = KT * U // 2
            split = {0, CPC - 1}
            w_ts = []
            for j in range(CPC):
                w_t = wp.tile([PT, KT * U], bf16, tag="w", name=f"w{j}")
                # Whole slots move as 128 x 4 KB lines (fastest per-engine
                # rate). The first and last slots go in halves so the PE
                # can start ~0.7 us earlier and only half of the final
                # slot's matmuls trail the last HBM byte.
                if j in split:
                    nc.sync.dma_start(w_t[:, :HW], wt[j][:, :HW])
                else:
                    nc.sync.dma_start(w_t[:], wt[j])
                if j == 0:
                    # x panel after w0's first half: the first matmuls
                    # need both, everything later only needs the weights.
                    xt_t = xp.tile([PT, KT * NCOL], bf16, name="x")
                    nc.sync.dma_start(xt_t[:], xt[:, :])
                if j in split:
                    nc.sync.dma_start(w_t[:, HW:], wt[j][:, HW:])
                w_ts.append(w_t)

            ots = [op.tile([PT, U], bf16, name=f"o{g}") for g in range(OG)]

            # The LDWEIGHTS half of a matmul only supports one sync wait.
            # Per class, a tiny dummy matmul reading only w_t absorbs the
            # weight-DMA wait on the PE engine, so the real matmuls below
            # carry at most the xt-DMA / PSUM-reuse wait. The dummies form
            # one accumulation group into the same scratch element so they
            # do not create PSUM WAW waits among themselves.
            scrps = apool.tile([2, 2], f32, name="abs")

            def dummy(j, col, start=False, stop=False):
                nc.tensor.matmul(
                    scrps[:],
                    w_ts[j][:, col : col + 2],
                    w_ts[j][:, col + 2 : col + 4],
                    start=start,
                    stop=stop,
                    skip_group_check=True,
                )

            U2 = U // 2
            stores = []
            for j in range(CPC):
                # A dummy is only needed where the i=0 LDWEIGHTS must carry
                # another wait besides the half-A DMA: class 0 (xt DMA) and
                # the PSUM-bank-reuse classes (ACT completion).
                if j == 0 or j >= GRP * 3:
                    dummy(j, 0, start=(j == 0), stop=(j == CPC - 1))
                g, r = divmod(j, GRP)
                if r == 0:
                    ps = pp.tile([PT, U], f32, tag="ps", name=f"ps{g}")
                rows = min(PT, NCOL - PT * g)
                if j < CPC - 1:
                    for i in range(KT):
                        nc.tensor.matmul(
                            ps[S * r : S * r + S, :],
                            xt_t[:, i * NCOL + S * j : i * NCOL + S * (j + 1)],
                            w_ts[j][:, i * U : (i + 1) * U],
                            start=(i == 0),
                            stop=(i == KT - 1),
                            # PE-array column offset = PSUM partition
                            # offset; auto-infer rejects 96, so pass it
                            # explicitly.
                            tile_position=(0, S * r),
                        )
                    # Drain the PSUM bank when its last class finishes: one
                    # ACTIVATE casts 4 classes fp32->bf16 into the out tile.
                    if r == GRP - 1:
                        nc.scalar.copy(ots[g][:rows, :], ps[:rows, :])
                        stores.append((g, rows))
                else:
                    # Final slot: its weights are laid out U-half-major on
                    # the host, so each DMA half completes a full K
                    # reduction for one U-half. The first half's drain and
                    # store overlap the second half's arrival and matmuls,
                    # shortening the after-last-HBM-byte tail.
                    for h in range(2):
                        for i in range(KT):
                            nc.tensor.matmul(
                                ps[S * r : S * r + S, h * U2 : (h + 1) * U2],
                                xt_t[
                                    :, i * NCOL + S * j : i * NCOL + S * (j + 1)
                                ],
                                w_ts[j][:, h * HW + i * U2 : h * HW + (i + 1) * U2],
                                start=(i == 0),
                                stop=(i == KT - 1),
                                tile_position=(0, S * r),
                            )
                        nc.scalar.copy(
                            ots[g][:rows, h * U2 : (h + 1) * U2],
                            ps[:rows, h * U2 : (h + 1) * U2],
                        )
                    stores.append((g, rows))
            # Early groups store from the Scalar queue mid-stream; the
            # final group stores from the Sync queue, which is empty by
            # then and moves the data immediately (the Scalar queue
            # trickles at ~35 GB/s when cold). Re-storing groups 0-1 from
            # the Sync queue first is an idempotent warm-up: their packets
            # sit in the queue behind the weight stream and execute right
            # as it drains, keeping the DMA engines awake until the final
            # store's descriptors arrive.
            for g, rows in stores[:-1]:
                nc.scalar.dma_start(
                    out[PT * g : PT * g + rows, :], ots[g][:rows, :]
                )
            for g, rows in stores[:2]:
                nc.sync.dma_start(
                    out[PT * g : PT * g + rows, :], ots[g][:rows, :]
                )
            # Final group: U-half-A from the Sync queue as soon as its
            # ACTIVATE lands, U-half-B from the (still warm) Scalar queue
            # in parallel.
            gl, rows = stores[-1]
            nc.sync.dma_start(
                out[PT * gl : PT * gl + rows, :U2], ots[gl][:rows, :U2]
            )
            nc.scalar.dma_start(
                out[PT * gl : PT * gl + rows, U2:], ots[gl][:rows, U2:]
            )
    _split_multi_waits(nc)
    return nc


def kernel(x, classes, kernel, bias):
    global LAST_RESULTS
    x = np.asarray(x, dtype=np.float32)
    W = np.asarray(kernel, dtype=np.float32)
    bias_np = np.asarray(bias, dtype=np.float32)
    cls = np.asarray(classes).reshape(-1).astype(np.int64)

    counts = np.bincount(cls, minlength=C)
    # Fixed column width per class slot; multiple of 8 for DMA alignment.
    S = int(max(32, -(-counts.max() // 8) * 8))
    if S not in _PROG_CACHE:
        _PROG_CACHE[S] = _build_program(S)
    nc = _PROG_CACHE[S]
    NCOL = CPC * S

    order = np.argsort(cls, kind="stable")
    starts = np.zeros(C + 1, np.int64)
    np.cumsum(counts[:C], out=starts[1:])
    rows_by_class = [order[starts[c] : starts[c + 1]] for c in range(C)]

    # Weight slots, pre-tiled to the SBUF layout: [slot, p, i*U+u] holds
    # W[c, i*128+p, u] so each DMA line is 4 KB contiguous.
    W_t = np.ascontiguousarray(
        W.reshape(C, KT, PT, U).transpose(0, 2, 1, 3).reshape(C, PT, KT * U)
    ).astype(BF16)

    in_maps = []
    for m in range(NCORES):
        xt_m = np.zeros((D, NCOL), np.float32)
        for j in range(CPC):
            c = m * CPC + j
            if c >= C:
                continue
            r = rows_by_class[c]
            if r.size:
                xt_m[:, S * j : S * j + r.size] = x[r].T
        # Pre-tile x panel: [p, i*NCOL + c] = xt[i*128+p, c].
        xt_dev = np.ascontiguousarray(
            xt_m.reshape(KT, PT, NCOL).transpose(1, 0, 2).reshape(PT, KT * NCOL)
        ).astype(BF16)
        if (m + 1) * CPC <= C:
            wt_m = np.array(W_t[m * CPC : (m + 1) * CPC])
            lc = m * CPC + CPC - 1
        else:
            n_real = C - m * CPC
            wt_m = np.concatenate([W_t[m * CPC : C], W_t[: CPC - n_real]], axis=0)
            lc = CPC - n_real - 1
        # Final slot is U-half-major: [p, h*HW + i*U2 + u] = W[lc, i*128+p,
        # h*U2+u], so each DMA half holds a full K reduction for one U-half.
        U2 = U // 2
        wt_m[CPC - 1] = (
            W[lc]
            .reshape(KT, PT, 2, U2)
            .transpose(1, 2, 0, 3)
            .reshape(PT, KT * U)
            .astype(BF16)
        )
        in_maps.append({"xt": xt_dev, "wt": np.ascontiguousarray(wt_m)})

    res = run_bass_kernel_spmd(nc, in_maps, list(range(NCORES)))
    LAST_RESULTS = res

    out = np.empty((B, U), np.float32)
    for m in range(NCORES):
        panel = np.asarray(res.results[m]["out"]).astype(np.float32)
        for j in range(CPC):
            c = m * CPC + j
            if c >= C:
                continue
            r = rows_by_class[c]
            if r.size:
                out[r] = panel[S * j : S * j + r.size] + bias_np[c]
    return out
